# revision 16
# baseline (speedup 1.0000x reference)
"""Trainium2 Bass kernel for the nn_Attention problem (B=4, S=1024, H=32, D=128).

Sharding: zero-collective data-parallel split. Each of the 8 cores owns one
(batch, half) pair: batch b = core//2, half = core%2. A half owns 4 of the 8
query blocks of 128 tokens, interleaved for causal balance:
  half 0 -> blocks [7, 5, 2, 0]   half 1 -> blocks [6, 4, 3, 1]
Query columns are packed in DESCENDING block order so that key-block t only
needs a PREFIX of the packed columns. Each core computes Q proj (its tokens,
all heads), K/V proj (its whole batch), causal attention and the full output
projection for its tokens, then the host scatters the 8 token-slices back
into the full [B, S, HID] output.

Schedule (v5):
- KV-FIRST front: the KV projection has the best PE-work per DMA-byte ratio
  (0.86us PE per 512KB hh pair), so it leads; hidq/wq0 DMAs are interleaved
  into the hh stream and Q-proj head-0 chunks are woven between KV pair
  matmuls to absorb the DMA-starved stretches. Pipeline depth is PRE=2
  (only q0 roped + q1 projected before steady state), which shrinks the
  drain from 5 heads to 2.
- Steady state h=2..31: attend head h-2, project head h, with the Q-proj
  matmuls split into filler chunks placed where the softmax chains (mask ->
  exp -> PV, exacc -> sums) would otherwise starve the PE: (0,4) after
  scores ph1, (4,8) woven between PV ph1, (8,12) after scores ph2, (12,16)
  woven between PV ph2, (16,24) before attn_end, (24,32) after the rope.
- The Q eviction + RoPE of head h are split across the iteration boundary
  so the partition-swap DMA round-trip never stalls the DVE stream.
- Drain: heads 30/31's softmax chains are covered by partial O-proj
  accumulations for m=1,0 woven at the same phase points.

On-chip layout is transposed ([feature, token]) so every matmul has the
contraction dim on partitions with no transposes in the hot path.
"""

import numpy as np
import ml_dtypes

import concourse.bass as bass
import concourse.tile as tile
from concourse import bacc, mybir
from concourse.bass_utils import run_bass_kernel_spmd

B, S, H, D = 4, 1024, 32, 128
HID = H * D          # 4096
ROT = D // 2         # 64
HR = ROT // 2        # 32
ROPE_BASE = 10000.0
P = 128
NT = 512             # query tokens per core
NCORES = 8
KT = HID // P        # 32 contraction tiles
NKB = S // P         # 8 key blocks
SCALE = float(D) ** -0.5
PRE = 2              # heads projected before the steady loop

BLOCKS = [[7, 5, 2, 0], [6, 4, 3, 1]]
# hid3 key columns are host-permuted so the packed query columns are a
# PREFIX of hid3 (hidq never touches HBM: it is copied on-chip out of the
# KV stream tiles). Key slot t then holds the t-th packed block of (own
# half ++ other half), giving the visibility profile below.
NVMAX = [128, 256, 384, 512, 128, 256, 384, 512]
MWIN = [(0, 128), (128, 256), (256, 384), (384, 512),
        (0, 128), (128, 256), (256, 384), (384, 512)]
TORD = [3, 0, 1, 2, 7, 4, 5, 6]      # slot processing order (full-width first)
T0, TL = TORD[0], TORD[-1]

BF = mybir.dt.bfloat16
F32 = mybir.dt.float32
AF = mybir.ActivationFunctionType
bf16 = ml_dtypes.bfloat16

_PROG = None


def _build():
    nc = bacc.Bacc("TRN2", target_bir_lowering=False, debug=False,
                   num_devices=NCORES)
    dp = nc.declare_dram_parameter
    hid3 = dp("hid3", [P, KT, S], BF, False)        # [p, k, t] = hidden[b,t,k*128+p]
    hidq = dp("hidq", [P, KT, NT], BF, False)       # packed query columns
    wq4 = dp("wq4", [H, P, KT, P], BF, False)       # [h,p,k,d] = Wq[h*128+d, k*128+p]
    wo4 = dp("wo4", [KT, P, KT, P], BF, False)      # [m,p,k,d] = Wo[m*128+d, k*128+p]
    wkv3 = dp("wkv3", [P, KT, 2 * D], BF, False)    # [p,k,c] = Wkv[c, k*128+p]
    bcat = dp("bcat", [P, H + KT + 2], F32, False)  # bq | bo | bkv packed
    csq = dp("csq", [ROT, 2, NT], BF, False)        # cos | sin (queries)
    csk = dp("csk", [ROT, 2, S], BF, False)         # cos | sin (keys)
    mcat = dp("mcat", [P, NKB + 1, P], BF, False)   # maskt blocks | ident
    outp = dp("out", [KT, P, NT], BF, True)        # [m, dd, c] = out.T slice

    with tile.TileContext(nc) as tc:
        with (
            tc.tile_pool(name="const", bufs=1) as constp,
            tc.tile_pool(name="persist", bufs=1) as persist,
            tc.tile_pool(name="wqp", bufs=4) as wqp,
            tc.tile_pool(name="qtmp", bufs=3) as qtmp,
            tc.tile_pool(name="qrp", bufs=4) as qrp,
            tc.tile_pool(name="expp", bufs=5) as expp,
            tc.tile_pool(name="exap", bufs=2) as exap,
            tc.tile_pool(name="nrm", bufs=2) as nrm,
            tc.tile_pool(name="outsb", bufs=2) as outsb,
        ):
            attn_all = persist.tile([P, KT, NT], BF, tag="attn")
            kbf = persist.tile([P, S], BF, tag="kbf")
            vnat = persist.tile([P, NKB, P], BF, tag="vnat")
            hidq_sb = persist.tile([P, KT, NT], BF, tag="hidq")

            # warm up the ScalarE activation table (the lazy 1.3us
            # ACT_TABLE_LOAD otherwise lands on the first eviction)
            scr = constp.tile([1, 1], F32, tag="scr")
            nc.gpsimd.memset(scr[:], 0.0)
            nc.scalar.activation(scr[:], scr[:], AF.Identity)
            ones_row = constp.tile([1, P], BF, tag="onesr")
            nc.gpsimd.memset(ones_row[:], 1.0)
            ones_sb = constp.tile([P, 1], BF, tag="ones")
            nc.gpsimd.memset(ones_sb[:], 1.0)

            with tc.tile_pool(name="psq", bufs=2, space="PSUM") as psq:

                def qproj_matmuls(qp, wq_sb, k0, k1):
                    for k in range(k0, k1):
                        nc.tensor.matmul(qp[:], wq_sb[:, k, :], hidq_sb[:, k, :],
                                         start=(k == 0), stop=(k == KT - 1))

                def qproj_evict(h, qp, swap_engine):
                    qfp = qtmp.tile([P, NT], BF, tag="qfp")
                    nc.scalar.activation(qfp[0:ROT, :], qp[0:ROT, :], AF.Identity,
                                         bias=bq_sb[0:ROT, h:h + 1])
                    qsw = qtmp.tile([ROT, NT], BF, tag="qsw")
                    swap_engine.dma_start(qsw[0:HR, :], qfp[HR:ROT, :])
                    swap_engine.dma_start(qsw[HR:ROT, :], qfp[0:HR, :])
                    nc.scalar.activation(qfp[ROT:P, :], qp[ROT:P, :], AF.Identity,
                                         bias=bq_sb[ROT:P, h:h + 1])
                    return qfp, qsw

                def qproj_rope(qfp, qsw):
                    m1q = qtmp.tile([ROT, NT], BF, tag="m1q")
                    m2q = qtmp.tile([ROT, NT], BF, tag="m2q")
                    qr = qrp.tile([P, NT], BF, tag="qr")
                    nc.vector.tensor_mul(m1q[:], qfp[0:ROT, :], cosq_sb[:])
                    nc.vector.tensor_mul(m2q[:], qsw[:], sinq_sb[:])
                    nc.vector.tensor_sub(qr[0:HR, :], m1q[0:HR, :], m2q[0:HR, :])
                    nc.vector.tensor_add(qr[HR:ROT, :], m1q[HR:ROT, :],
                                         m2q[HR:ROT, :])
                    nc.vector.tensor_copy(qr[ROT:P, :], qfp[ROT:P, :])
                    return qr

                qrs = {}

                # ================= KV-first front =================
                with (
                    tc.tile_pool(name="hidp", bufs=4) as hidp,
                    tc.tile_pool(name="wkvp", bufs=1) as wkvp,
                    tc.tile_pool(name="kvtmp", bufs=1) as kvtmp,
                    tc.tile_pool(name="pskv", bufs=1, space="PSUM") as pskv,
                    tc.tile_pool(name="pstr", bufs=2, space="PSUM") as pstr,
                ):
                    kps = [pskv.tile([P, S // 2], F32, name=f"kps{j}",
                                     tag=f"kps{j}") for j in range(2)]
                    vps = [pskv.tile([P, S // 2], F32, name=f"vps{j}",
                                     tag=f"vps{j}") for j in range(2)]

                    # ---- DMA stream: KV data leads; few BIG descriptors
                    # (each dma_start costs ~0.5-0.8us of descriptor
                    # generation on the issuing queue, so many small DMAs
                    # serialize the front) ----
                    wkv_all = wkvp.tile([P, KT, 2 * D], BF, tag="wkvall")
                    w0 = wqp.tile([P, KT, P], BF, tag="w")
                    nc.sync.dma_start(wkv_all[:], wkv3[:])
                    hh = []

                    def hh_quad():
                        t = hidp.tile([P, 4, S], BF, tag="hh")
                        nc.sync.dma_start(t[:], hid3[:, 4 * len(hh):
                                                      4 * len(hh) + 4, :])
                        hh.append(t)

                    hh_quad()                                   # q0
                    nc.sync.dma_start(hidq_sb[:, 0:16, :], hidq[:, 0:16, :])
                    hh_quad()                                   # q1
                    nc.sync.dma_start(w0[:], wq4[0])
                    hh_quad()                                   # q2
                    nc.sync.dma_start(hidq_sb[:, 16:32, :], hidq[:, 16:32, :])
                    hh_quad()                                   # q3
                    w1 = wqp.tile([P, KT, P], BF, tag="w")
                    nc.sync.dma_start(w1[:], wq4[1])
                    bcat_sb = constp.tile([P, H + KT + 2], F32, tag="bcat")
                    nc.sync.dma_start(bcat_sb[:], bcat[:])
                    bq_sb = bcat_sb[:, 0:H]
                    bo_sb = bcat_sb[:, H:H + KT]
                    bkv_sb = bcat_sb[:, H + KT:H + KT + 2]
                    hh_quad()                                   # q4
                    csk_sb = constp.tile([ROT, 2, S], BF, tag="csk")
                    nc.sync.dma_start(csk_sb[:], csk[:])
                    cosk_sb = csk_sb[:, 0, :]
                    sink_sb = csk_sb[:, 1, :]
                    hh_quad()                                   # q5
                    csq_sb = constp.tile([ROT, 2, NT], BF, tag="csq")
                    nc.sync.dma_start(csq_sb[:], csq[:])
                    cosq_sb = csq_sb[:, 0, :]
                    sinq_sb = csq_sb[:, 1, :]
                    hh_quad()                                   # q6
                    mcat_sb = constp.tile([P, NKB + 1, P], BF, tag="mcat")
                    nc.sync.dma_start(mcat_sb[:], mcat[:])
                    mask_sb = mcat_sb[:, 0:NKB, :]
                    id_sb = mcat_sb[:, NKB, :]
                    hh_quad()                                   # q7

                    # ---- KV matmuls with Q-proj head-0 chunks woven in ----
                    qp0 = psq.tile([P, NT], F32, tag="qp")

                    def kv_quad(q):
                        for kk in range(4 * q, 4 * q + 4):
                            hhk = hh[q][:, kk - 4 * q, :]
                            stk = (kk == 0)
                            spk = (kk == KT - 1)
                            for j in range(2):
                                nc.tensor.matmul(kps[j][:], wkv_all[:, kk, 0:D],
                                                 hhk[:, j * 512:(j + 1) * 512],
                                                 start=stk, stop=spk)
                            for j in range(2):
                                nc.tensor.matmul(vps[j][:],
                                                 wkv_all[:, kk, D:2 * D],
                                                 hhk[:, j * 512:(j + 1) * 512],
                                                 start=stk, stop=spk)

                    def hidq_copy(q):
                        pass

                    kv_quad(0)
                    hidq_copy(0)
                    kv_quad(1)
                    hidq_copy(1)
                    qproj_matmuls(qp0, w0, 0, 8)
                    kv_quad(2)
                    hidq_copy(2)
                    qproj_matmuls(qp0, w0, 8, 16)
                    kv_quad(3)
                    hidq_copy(3)
                    qproj_matmuls(qp0, w0, 16, 24)
                    kv_quad(4)
                    hidq_copy(4)
                    qproj_matmuls(qp0, w0, 24, KT)
                    kv_quad(5)
                    hidq_copy(5)
                    kv_quad(6)
                    hidq_copy(6)
                    kv_quad(7)
                    hidq_copy(7)

                    # ---- KV epilogue: evictions, K RoPE, V PE-transpose ----
                    kfp = kvtmp.tile([P, S], BF, tag="kfp")
                    vbf = kvtmp.tile([P, S], BF, tag="vbf")
                    for j in range(2):
                        sl = slice(j * 512, (j + 1) * 512)
                        nc.scalar.activation(kfp[:, sl], kps[j][:], AF.Identity,
                                             bias=bkv_sb[:, 0:1])
                        nc.scalar.activation(vbf[:, sl], vps[j][:], AF.Identity,
                                             bias=bkv_sb[:, 1:2])
                    ksw = kvtmp.tile([ROT, S], BF, tag="ksw")
                    nc.scalar.dma_start(ksw[0:HR, :], kfp[HR:ROT, :])
                    nc.scalar.dma_start(ksw[HR:ROT, :], kfp[0:HR, :])
                    m1 = kvtmp.tile([ROT, S], BF, tag="m1")
                    m2 = kvtmp.tile([ROT, S], BF, tag="m2")
                    nc.vector.tensor_mul(m1[:], kfp[0:ROT, :], cosk_sb[:])
                    nc.vector.tensor_mul(m2[:], ksw[:], sink_sb[:])
                    nc.vector.tensor_sub(kbf[0:HR, :], m1[0:HR, :], m2[0:HR, :])
                    nc.vector.tensor_add(kbf[HR:ROT, :], m1[HR:ROT, :],
                                         m2[HR:ROT, :])
                    nc.vector.tensor_copy(kbf[ROT:P, :], kfp[ROT:P, :])
                    for t in range(NKB):
                        vt = pstr.tile([P, P], BF, tag="vt")
                        nc.tensor.transpose(vt[:], vbf[:, t * P:(t + 1) * P],
                                            id_sb[:])
                        nc.vector.tensor_copy(vnat[:, t, :], vt[:])

                    # ---- finish head 0 (evict+rope inline), project head 1
                    ev0 = qproj_evict(0, qp0, nc.scalar)
                    qrs[0] = qproj_rope(*ev0)
                    qp1 = psq.tile([P, NT], F32, tag="qp")
                    qproj_matmuls(qp1, w1, 0, KT)
                    pending = (1, qp1)

                with (
                    tc.tile_pool(name="pss", bufs=3, space="PSUM") as pss,
                    tc.tile_pool(name="pso", bufs=2, space="PSUM") as pso,
                    tc.tile_pool(name="pssum", bufs=1, space="PSUM") as pssum,
                ):
                    def attn_scores(qr, trange):
                        sps = []
                        for t in trange:
                            nv = NVMAX[t]
                            sp = pss.tile([P, NT], F32, tag="sp")
                            nc.tensor.matmul(sp[:, 0:nv],
                                             kbf[:, t * P:(t + 1) * P],
                                             qr[:, 0:nv], start=True, stop=True)
                            sps.append((t, sp))
                        for t, sp in sps:
                            lo, hi = MWIN[t]
                            nc.vector.tensor_add(sp[:, lo:hi], sp[:, lo:hi],
                                                 mask_sb[:, t, :])
                        return sps

                    def attn_pv(sps, op, fillers=()):
                        """exp + PV matmuls; fillers: per-PV-matmul PE filler
                        callables woven between the PV matmuls."""
                        exs = []
                        for t, sp in sps:
                            ex = expp.tile([P, NT], BF, tag="ex")
                            nc.scalar.activation(ex[:, 0:NVMAX[t]],
                                                 sp[:, 0:NVMAX[t]], AF.Exp,
                                                 scale=SCALE)
                            exs.append((t, ex))
                        fi = list(fillers)
                        for i, (t, ex) in enumerate(exs):
                            nc.tensor.matmul(op[:, 0:NVMAX[t]], vnat[:, t, :],
                                             ex[:, 0:NVMAX[t]],
                                             start=(t == T0), stop=(t == TL))
                            if i < len(fi):
                                fi[i]()
                        for f in fi[len(exs):]:
                            f()
                        return exs

                    def attn_exacc(exacc, exs):
                        for t, ex in exs:
                            if t == T0:
                                # T0 is full-width: its copy initializes all
                                # NT columns of the accumulator
                                nc.vector.tensor_copy(exacc[:], ex[:])
                            else:
                                nc.vector.tensor_add(exacc[:, 0:NVMAX[t]],
                                                     exacc[:, 0:NVMAX[t]],
                                                     ex[:, 0:NVMAX[t]])

                    def attn_end(h, op, exacc, pe_norm=False):
                        sums = pssum.tile([1, NT], F32, tag="sums")
                        nc.tensor.matmul(sums[:], ones_sb[:], exacc[:],
                                         start=True, stop=True)
                        rs = nrm.tile([1, NT], F32, tag="rs")
                        nc.vector.reciprocal_approx_fast(rs[:], sums[:])
                        if pe_norm:
                            # drain heads: broadcast 1/sums on the PE instead
                            # of gpsimd (whose op+drain is ~2us of exposed
                            # critical path once there is no filler left)
                            rsb = nrm.tile([1, NT], BF, tag="rsb")
                            nc.vector.tensor_copy(rsb[:], rs[:])
                            rb_ps = pssum.tile([P, NT], F32, tag="sums")
                            nc.tensor.matmul(rb_ps[:], ones_row[:], rsb[:],
                                             start=True, stop=True)
                            rb = nrm.tile([P, NT], F32, tag="rb")
                            nc.vector.tensor_copy(rb[:], rb_ps[:])
                        else:
                            rb = nrm.tile([P, NT], F32, tag="rb")
                            nc.gpsimd.partition_broadcast(rb[:], rs[:])
                        nc.vector.tensor_mul(attn_all[:, h, :], op[:], rb[:])

                    # ---- steady-state head pipeline (depth PRE=2) ----
                    # scores run in 3 phases of {3,3,2} key blocks so pss
                    # needs only 3 PSUM banks, freeing one for pso bufs=2:
                    # the next head's PV can then accumulate into the other
                    # op bank while this head's normalization chain
                    # (exacc->sums->recip->bcast->mul) completes leisurely.
                    PH = [TORD[0:3], TORD[3:6], TORD[6:NKB]]

                    def head_attn(ha, qr_a, op, exacc, fill):
                        """fill: 8 callables woven at the phase points."""
                        sps = attn_scores(qr_a, PH[0])
                        fill[0]()
                        exs = attn_pv(sps, op, fillers=[fill[1]])
                        attn_exacc(exacc, exs)
                        sps = attn_scores(qr_a, PH[1])
                        fill[2]()
                        exs = attn_pv(sps, op, fillers=[fill[3]])
                        attn_exacc(exacc, exs)
                        sps = attn_scores(qr_a, PH[2])
                        fill[4]()
                        exs = attn_pv(sps, op, fillers=[fill[5]])
                        attn_exacc(exacc, exs)
                        fill[6]()

                    for h in range(PRE, H):
                        wq_sb = wqp.tile([P, KT, P], BF, tag="w")
                        nc.sync.dma_start(wq_sb[:], wq4[h])
                        qp = psq.tile([P, NT], F32, tag="qp")
                        ha = h - PRE
                        qr_a = qrs.pop(ha)
                        op = pso.tile([P, NT], F32, tag="op")
                        exacc = exap.tile([P, NT], BF, tag="exacc")
                        hp, qpp = pending
                        ev = qproj_evict(hp, qpp, nc.sync)

                        def qch(k0, k1):
                            return lambda: qproj_matmuls(qp, wq_sb, k0, k1)

                        head_attn(ha, qr_a, op, exacc,
                                  [qch(0, 3), qch(3, 6), qch(6, 9), qch(9, 12),
                                   qch(12, 16), qch(16, 18), qch(18, 24)])
                        attn_end(ha, op, exacc)
                        qrs[hp] = qproj_rope(*ev)
                        qproj_matmuls(qp, wq_sb, 24, KT)
                        pending = (h, qp)

                    # finish the last projected head (31)
                    ev = qproj_evict(pending[0], pending[1], nc.sync)
                    qrs[pending[0]] = qproj_rope(*ev)

                    # ---- drain: attend heads 30, 31 with O-proj m=1,0
                    # partial accumulations woven at the phase points ----
                    wo_sb1 = wqp.tile([P, KT, P], BF, tag="w")
                    nc.sync.dma_start(wo_sb1[:], wo4[1])
                    wo_sb0 = wqp.tile([P, KT, P], BF, tag="w")
                    nc.sync.dma_start(wo_sb0[:], wo4[0])
                    fp1 = psq.tile([P, NT], F32, tag="qp")
                    fp0 = psq.tile([P, NT], F32, tag="qp")

                    def omm(fp, wo_sb, k0, k1, stop=False):
                        def f():
                            for k in range(k0, k1):
                                nc.tensor.matmul(fp[:], wo_sb[:, k, :],
                                                 attn_all[:, k, :],
                                                 start=(k == 0),
                                                 stop=(stop and k == k1 - 1))
                        return f

                    def emit_attn_f(h, qr, fill, pe_norm=True):
                        op = pso.tile([P, NT], F32, tag="op")
                        exacc = exap.tile([P, NT], BF, tag="exacc")
                        head_attn(h, qr, op, exacc, fill)
                        attn_end(h, op, exacc, pe_norm)

                    emit_attn_f(H - 2, qrs.pop(H - 2),
                                [omm(fp1, wo_sb1, 0, 3), omm(fp1, wo_sb1, 3, 6),
                                 omm(fp1, wo_sb1, 6, 9),
                                 omm(fp1, wo_sb1, 9, 12),
                                 omm(fp1, wo_sb1, 12, 15),
                                 omm(fp1, wo_sb1, 15, 18), lambda: None])
                    emit_attn_f(H - 1, qrs.pop(H - 1),
                                [omm(fp0, wo_sb0, 0, 3), omm(fp0, wo_sb0, 3, 6),
                                 omm(fp0, wo_sb0, 6, 9),
                                 omm(fp0, wo_sb0, 9, 12),
                                 omm(fp0, wo_sb0, 12, 15),
                                 omm(fp0, wo_sb0, 15, 18), lambda: None])
                    omm(fp1, wo_sb1, 18, KT, stop=True)()
                    ob1 = outsb.tile([P, NT], BF, tag="ob")
                    nc.scalar.activation(ob1[:], fp1[:], AF.Identity,
                                         bias=bo_sb[:, 1:2])
                    nc.sync.dma_start(outp[1], ob1[:])
                    omm(fp0, wo_sb0, 18, KT, stop=True)()
                    ob0 = outsb.tile([P, NT], BF, tag="ob")
                    nc.scalar.activation(ob0[:], fp0[:], AF.Identity,
                                         bias=bo_sb[:, 0:1])
                    nc.sync.dma_start(outp[0], ob0[:])

                    # ---- output projection (rest) ----
                    for m in range(2, KT):
                        wo_sb = wqp.tile([P, KT, P], BF, tag="w")
                        nc.sync.dma_start(wo_sb[:], wo4[m])
                        fp = psq.tile([P, NT], F32, tag="qp")
                        for k in range(KT):
                            nc.tensor.matmul(fp[:], wo_sb[:, k, :],
                                             attn_all[:, k, :],
                                             start=(k == 0), stop=(k == KT - 1))
                        ob = outsb.tile([P, NT], BF, tag="ob")
                        if m == KT - 1:
                            # quarter the final eviction->writeback drain;
                            # alternate desc queues so desc-gen overlaps
                            for q in range(4):
                                sl = slice(q * NT // 4, (q + 1) * NT // 4)
                                nc.scalar.activation(ob[:, sl], fp[:, sl],
                                                     AF.Identity,
                                                     bias=bo_sb[:, m:m + 1])
                                eng = nc.sync if q % 2 == 0 else nc.gpsimd
                                eng.dma_start(outp[m, :, sl], ob[:, sl])
                        else:
                            nc.scalar.activation(ob[:], fp[:], AF.Identity,
                                                 bias=bo_sb[:, m:m + 1])
                            nc.sync.dma_start(outp[m], ob[:])

    nc.compile()
    return nc


def _get_prog():
    global _PROG
    if _PROG is None:
        _PROG = _build()
    return _PROG


def _qcols(hf):
    return np.concatenate([np.arange(b * P, (b + 1) * P) for b in BLOCKS[hf]])


def _prepare_inmaps(inputs):
    pos = np.asarray(inputs["position_ids"])
    hs = np.asarray(inputs["hidden_states"], np.float32)
    Wq = np.asarray(inputs["Wq"], np.float32)
    bq = np.asarray(inputs["bq"], np.float32)
    Wkv = np.asarray(inputs["Wkv"], np.float32)
    bkv = np.asarray(inputs["bkv"], np.float32)
    Wo = np.asarray(inputs["Wo"], np.float32)
    bo = np.asarray(inputs["bo"], np.float32)

    shared = {
        "wq4": np.ascontiguousarray(
            Wq.reshape(H, P, KT, P).transpose(0, 3, 2, 1)).astype(bf16),
        "wo4": np.ascontiguousarray(
            Wo.reshape(KT, P, KT, P).transpose(0, 3, 2, 1)).astype(bf16),
        "wkv3": np.ascontiguousarray(
            Wkv.T.reshape(KT, P, 2 * D).transpose(1, 0, 2)).astype(bf16),
        "bcat": np.ascontiguousarray(np.concatenate(
            [bq.reshape(H, P).T, bo.reshape(KT, P).T,
             bkv.reshape(2, P).T], axis=1)),
    }

    invf = (1.0 / (ROPE_BASE ** (np.arange(0, ROT, 2, dtype=np.float32)
                                 / np.float32(ROT)))).astype(np.float32)
    in_maps = []
    for c in range(NCORES):
        b, hf = c // 2, c % 2
        qc = _qcols(hf)
        order = np.concatenate([qc, _qcols(1 - hf)])  # key col permutation
        posb = pos[b].astype(np.float32)
        ang = invf[:, None] * posb[None, :]          # [32, S]
        cos1 = np.cos(ang).astype(np.float32)
        sin1 = np.sin(ang).astype(np.float32)
        cos2k = np.concatenate([cos1, cos1], 0)[:, order]  # [64, S] permuted
        sin2k = np.concatenate([sin1, sin1], 0)[:, order]
        hidT = np.ascontiguousarray(hs[b].T[:, order])  # [HID, S] permuted
        # Causal mask over sequence INDICES (jnp.tril in the reference);
        # key slot t*128+p holds sequence index order[t*128+p].
        kseq = order.reshape(1, NKB, P).transpose(2, 1, 0)  # [P, NKB, 1]
        mask = np.where(kseq <= qc[None, None, :], 0.0, -1e30).astype(np.float32)
        for t in range(NKB):
            lo, hi = MWIN[t]
            assert not mask[:, t, :lo].any() and not mask[:, t, hi:NVMAX[t]].any(), \
                f"mask outside window at t={t}"
        maskc = np.stack([mask[:, t, MWIN[t][0]:MWIN[t][1]]
                          for t in range(NKB)], axis=1)   # [P, NKB, 128]
        m = dict(shared)
        m["hid3"] = np.ascontiguousarray(
            hidT.reshape(KT, P, S).transpose(1, 0, 2)).astype(bf16)
        m["hidq"] = np.ascontiguousarray(
            hidT[:, 0:NT].reshape(KT, P, NT).transpose(1, 0, 2)).astype(bf16)
        m["csq"] = np.ascontiguousarray(
            np.stack([cos2k[:, 0:NT], sin2k[:, 0:NT]], axis=1)).astype(bf16)
        m["csk"] = np.ascontiguousarray(
            np.stack([cos2k, sin2k], axis=1)).astype(bf16)
        m["mcat"] = np.ascontiguousarray(np.concatenate(
            [maskc, np.eye(P, dtype=np.float32)[:, None, :]],
            axis=1)).astype(bf16)
        in_maps.append(m)
    return in_maps


def _assemble(results):
    out = np.empty((B, S, HID), np.float32)
    for c in range(NCORES):
        b, hf = c // 2, c % 2
        outT = np.asarray(results[c]["out"], np.float32).reshape(HID, NT)
        out[b, _qcols(hf), :] = outT.T
    return out


def _run(inputs, trace=False, **kw):
    nc = _get_prog()
    in_maps = _prepare_inmaps(inputs)
    try:
        res = run_bass_kernel_spmd(nc, in_maps, list(range(NCORES)),
                                   trace=trace, **kw)
    except Exception:
        # transient device wedge (e.g. NRT_EXEC_UNIT_UNRECOVERABLE) — retry once
        res = run_bass_kernel_spmd(nc, in_maps, list(range(NCORES)),
                                   trace=trace, **kw)
    return _assemble(res.results), res


def kernel(**inputs):
    out, _ = _run(inputs)
    return out
